# revision 2
# baseline (speedup 1.0000x reference)
"""Trainium2 Bass kernel for nn_ChannelWisePatchLevelObfuscator.

Per-patch 256x256 dense obfuscation matmul + bias + tanh + channel permutation.
Sharding: data-parallel over batch B=64 across 8 NeuronCores (8 images/core);
weights/biases replicated. Host packs x into a group-sorted, pixel-major layout
so device DMAs are fully contiguous 1 MiB slabs; the device does the matmuls
(PE), bias+tanh (ACT); host scatters patches back to image layout and applies
the channel permutation while assembling the full output.
"""
import sys
import numpy as np

sys.path.insert(0, "/opt/trn_rl_repo")

import concourse.bacc as bacc  # noqa: E402
import concourse.mybir as mybir  # noqa: E402
import concourse.tile as tile  # noqa: E402
from concourse.bass_utils import run_bass_kernel_spmd  # noqa: E402

IMG, C, PS, G, B = 512, 3, 16, 32, 64
NH = NW = IMG // PS          # 32
P2 = PS * PS                 # 256
NCORES = 8
BS = B // NCORES             # 8 images per core
T = BS * NH                  # 256 matmul rows per (c, g)
GB = 4                       # groups per SBUF block (1 MiB tiles)
NGB = G // GB                # 8 blocks per channel

F32 = mybir.dt.float32
MM_DT = mybir.dt.float32     # matmul input dtype (float32 | float32r)

_g = np.arange(G)[:, None]
_r = np.arange(NH)[None, :]
COLS = (_g - _r) % NW        # (g, r) -> patch column for that group

_CACHE = {}


def _build_nc():
    nc = bacc.Bacc("TRN2", target_bir_lowering=False, debug=False,
                   num_devices=NCORES)
    # slab layouts: [c, gb, 128, free] so each (c, gb) tile load/store is one
    # contiguous 8 KiB descriptor per partition.
    xt = nc.dram_tensor("xt", [C, NGB, 128, GB * 2 * T], MM_DT,
                        kind="ExternalInput")
    w = nc.dram_tensor("w", [C, NGB, 128, GB * 2 * P2], MM_DT,
                       kind="ExternalInput")
    bias = nc.dram_tensor("bias", [128, C * G * 2], F32, kind="ExternalInput")
    out = nc.dram_tensor("out", [C, NGB, 128, GB * 2 * T], F32,
                         kind="ExternalOutput")

    with tile.TileContext(nc) as tc:
        with tc.tile_pool(name="biasp", bufs=1) as bias_pool, \
             tc.tile_pool(name="xtp", bufs=3) as xt_pool, \
             tc.tile_pool(name="wp", bufs=3) as w_pool, \
             tc.tile_pool(name="outp", bufs=3) as out_pool, \
             tc.tile_pool(name="psp", bufs=8, space="PSUM") as ps_pool:
            bias_sb = bias_pool.tile([128, C * G * 2], F32)
            nc.sync.dma_start(bias_sb[:], bias[:, :])
            for c in range(C):
                for gb in range(NGB):
                    xt_t = xt_pool.tile([128, GB * 2 * T], MM_DT)
                    nc.sync.dma_start(xt_t[:], xt[c, gb])
                    w_t = w_pool.tile([128, GB * 2 * P2], MM_DT)
                    nc.sync.dma_start(w_t[:], w[c, gb])
                    out_t = out_pool.tile([128, GB * 2 * T], F32)
                    for gl in range(GB):
                        for oc in range(2):
                            ps = ps_pool.tile([128, T], F32)
                            for kc in range(2):
                                base = (gl * 2 + kc) * P2
                                nc.tensor.matmul(
                                    ps[:],
                                    w_t[:, base + oc * 128: base + oc * 128 + 128],
                                    xt_t[:, (gl * 2 + kc) * T: (gl * 2 + kc + 1) * T],
                                    start=(kc == 0), stop=(kc == 1))
                            bidx = (c * G + gb * GB + gl) * 2 + oc
                            nc.scalar.activation(
                                out_t[:, (gl * 2 + oc) * T: (gl * 2 + oc + 1) * T],
                                ps[:],
                                mybir.ActivationFunctionType.Tanh,
                                bias=bias_sb[:, bidx: bidx + 1],
                                scale=1.0)
                    nc.sync.dma_start(out[c, gb], out_t[:])
    nc.compile()
    return nc


def _pack_xt(x_shard):
    # (BS, C, 512, 512) -> xt[c, gb, k_lo, (g_lo, kc, t)] slab layout
    xp = x_shard.reshape(BS, C, NH, PS, NW, PS)        # b c r py cl px
    sel = xp[:, :, _r, :, COLS, :]                     # g r b c py px
    xt = sel.transpose(3, 0, 4, 5, 2, 1).reshape(C, G, P2, T)
    # [c, g, p, t] -> [c, gb, k_lo, g_lo, kc, t]
    xt = xt.reshape(C, NGB, GB, 2, 128, T).transpose(0, 1, 4, 2, 3, 5)
    return np.ascontiguousarray(xt.reshape(C, NGB, 128, GB * 2 * T))


def _pack_w(w_full):
    # [c, g, p_in, p_out] -> [c, gb, k_lo, (g_lo, kc, o)]
    w2 = w_full.reshape(C, NGB, GB, 2, 128, P2).transpose(0, 1, 4, 2, 3, 5)
    return np.ascontiguousarray(w2.reshape(C, NGB, 128, GB * 2 * P2))


def _unpack_out(out_dev, dst, perm):
    # out_dev[c, gb, o_lo, (g_lo, oc, t)] -> dst[b, c_final, H, W] + perm
    od = out_dev.reshape(C, NGB, 128, GB, 2, T).transpose(0, 1, 3, 4, 2, 5)
    o = od.reshape(C, G, P2, BS, NH)                   # c g o b r
    src = o.transpose(1, 4, 3, 0, 2).reshape(G, NH, BS, C, PS, PS)
    tmp = np.empty((NH, NW, BS, C, PS, PS), dtype=out_dev.dtype)
    tmp[_r, COLS] = src                                # tmp[r, (g-r)%32] = src[g, r]
    img = tmp.transpose(2, 3, 0, 4, 1, 5).reshape(BS, C, IMG, IMG)
    dst[:] = img[:, perm]


def kernel(x, obfuscation_weights, obfuscation_biases, channel_permutation):
    x = np.ascontiguousarray(x, dtype=np.float32)
    w = np.ascontiguousarray(obfuscation_weights, dtype=np.float32)
    bias = np.asarray(obfuscation_biases, dtype=np.float32)
    perm = np.asarray(channel_permutation, dtype=np.int64)

    if "nc" not in _CACHE:
        _CACHE["nc"] = _build_nc()
    nc = _CACHE["nc"]

    bias_t = np.ascontiguousarray(
        bias.reshape(C, G, 2, 128).transpose(3, 0, 1, 2).reshape(128, C * G * 2))
    w_packed = _pack_w(w)

    in_maps = []
    for core in range(NCORES):
        xt = _pack_xt(x[core * BS:(core + 1) * BS])
        in_maps.append({"xt": xt, "w": w_packed, "bias": bias_t})

    res = run_bass_kernel_spmd(nc, in_maps, core_ids=list(range(NCORES)))
    _CACHE["last_results"] = res

    out = np.empty((B, C, IMG, IMG), dtype=np.float32)
    for core in range(NCORES):
        _unpack_out(res.results[core]["out"],
                    out[core * BS:(core + 1) * BS], perm)
    return out


# revision 3
# speedup vs baseline: 1.0991x; 1.0991x over previous
"""Trainium2 Bass kernel for nn_ChannelWisePatchLevelObfuscator.

Per-patch 256x256 dense obfuscation matmul + bias + tanh + channel permutation.
Sharding: data-parallel over batch B=64 across 8 NeuronCores (8 images/core);
weights/biases replicated. Host packs x into a group-sorted, pixel-major layout
so device DMAs are fully contiguous 1 MiB slabs; the device does the matmuls
(PE), bias+tanh (ACT); host scatters patches back to image layout and applies
the channel permutation while assembling the full output.
"""
import sys
import numpy as np

sys.path.insert(0, "/opt/trn_rl_repo")

import concourse.bacc as bacc  # noqa: E402
import concourse.mybir as mybir  # noqa: E402
import concourse.tile as tile  # noqa: E402
from concourse.bass_utils import run_bass_kernel_spmd  # noqa: E402

IMG, C, PS, G, B = 512, 3, 16, 32, 64
NH = NW = IMG // PS          # 32
P2 = PS * PS                 # 256
NCORES = 8
BS = B // NCORES             # 8 images per core
T = BS * NH                  # 256 matmul rows per (c, g)
GB = 4                       # groups per SBUF block (1 MiB tiles)
NGB = G // GB                # 8 blocks per channel

F32 = mybir.dt.float32
MM_DT = mybir.dt.float32r    # matmul input dtype (float32 | float32r)

_g = np.arange(G)[:, None]
_r = np.arange(NH)[None, :]
COLS = (_g - _r) % NW        # (g, r) -> patch column for that group

_CACHE = {}


def _build_nc():
    nc = bacc.Bacc("TRN2", target_bir_lowering=False, debug=False,
                   num_devices=NCORES)
    # slab layouts: [c, gb, 128, free] so each (c, gb) tile load/store is one
    # contiguous 8 KiB descriptor per partition.
    xt = nc.dram_tensor("xt", [C, NGB, 128, GB * 2 * T], MM_DT,
                        kind="ExternalInput")
    w = nc.dram_tensor("w", [C, NGB, 128, GB * 2 * P2], MM_DT,
                       kind="ExternalInput")
    bias = nc.dram_tensor("bias", [128, C * G * 2], F32, kind="ExternalInput")
    out = nc.dram_tensor("out", [C, NGB, 128, GB * 2 * T], F32,
                         kind="ExternalOutput")

    with tile.TileContext(nc) as tc:
        with tc.tile_pool(name="biasp", bufs=1) as bias_pool, \
             tc.tile_pool(name="xtp", bufs=3) as xt_pool, \
             tc.tile_pool(name="wp", bufs=3) as w_pool, \
             tc.tile_pool(name="outp", bufs=3) as out_pool, \
             tc.tile_pool(name="psp", bufs=8, space="PSUM") as ps_pool:
            bias_sb = bias_pool.tile([128, C * G * 2], F32)
            nc.sync.dma_start(bias_sb[:], bias[:, :])
            for c in range(C):
                for gb in range(NGB):
                    xt_t = xt_pool.tile([128, GB * 2 * T], MM_DT)
                    nc.sync.dma_start(xt_t[:], xt[c, gb])
                    w_t = w_pool.tile([128, GB * 2 * P2], MM_DT)
                    nc.sync.dma_start(w_t[:], w[c, gb])
                    out_t = out_pool.tile([128, GB * 2 * T], F32)
                    for gl in range(GB):
                        for oc in range(2):
                            ps = ps_pool.tile([128, T], F32)
                            for kc in range(2):
                                base = (gl * 2 + kc) * P2
                                nc.tensor.matmul(
                                    ps[:],
                                    w_t[:, base + oc * 128: base + oc * 128 + 128],
                                    xt_t[:, (gl * 2 + kc) * T: (gl * 2 + kc + 1) * T],
                                    start=(kc == 0), stop=(kc == 1))
                            bidx = (c * G + gb * GB + gl) * 2 + oc
                            nc.scalar.activation(
                                out_t[:, (gl * 2 + oc) * T: (gl * 2 + oc + 1) * T],
                                ps[:],
                                mybir.ActivationFunctionType.Tanh,
                                bias=bias_sb[:, bidx: bidx + 1],
                                scale=1.0)
                    nc.sync.dma_start(out[c, gb], out_t[:])
    nc.compile()
    return nc


def _pack_xt(x_shard):
    # (BS, C, 512, 512) -> xt[c, gb, k_lo, (g_lo, kc, t)] slab layout
    xp = x_shard.reshape(BS, C, NH, PS, NW, PS)        # b c r py cl px
    sel = xp[:, :, _r, :, COLS, :]                     # g r b c py px
    xt = sel.transpose(3, 0, 4, 5, 2, 1).reshape(C, G, P2, T)
    # [c, g, p, t] -> [c, gb, k_lo, g_lo, kc, t]
    xt = xt.reshape(C, NGB, GB, 2, 128, T).transpose(0, 1, 4, 2, 3, 5)
    return np.ascontiguousarray(xt.reshape(C, NGB, 128, GB * 2 * T))


def _pack_w(w_full):
    # [c, g, p_in, p_out] -> [c, gb, k_lo, (g_lo, kc, o)]
    w2 = w_full.reshape(C, NGB, GB, 2, 128, P2).transpose(0, 1, 4, 2, 3, 5)
    return np.ascontiguousarray(w2.reshape(C, NGB, 128, GB * 2 * P2))


def _unpack_out(out_dev, dst, perm):
    # out_dev[c, gb, o_lo, (g_lo, oc, t)] -> dst[b, c_final, H, W] + perm
    od = out_dev.reshape(C, NGB, 128, GB, 2, T).transpose(0, 1, 3, 4, 2, 5)
    o = od.reshape(C, G, P2, BS, NH)                   # c g o b r
    src = o.transpose(1, 4, 3, 0, 2).reshape(G, NH, BS, C, PS, PS)
    tmp = np.empty((NH, NW, BS, C, PS, PS), dtype=out_dev.dtype)
    tmp[_r, COLS] = src                                # tmp[r, (g-r)%32] = src[g, r]
    img = tmp.transpose(2, 3, 0, 4, 1, 5).reshape(BS, C, IMG, IMG)
    dst[:] = img[:, perm]


def kernel(x, obfuscation_weights, obfuscation_biases, channel_permutation):
    x = np.ascontiguousarray(x, dtype=np.float32)
    w = np.ascontiguousarray(obfuscation_weights, dtype=np.float32)
    bias = np.asarray(obfuscation_biases, dtype=np.float32)
    perm = np.asarray(channel_permutation, dtype=np.int64)

    if "nc" not in _CACHE:
        _CACHE["nc"] = _build_nc()
    nc = _CACHE["nc"]

    bias_t = np.ascontiguousarray(
        bias.reshape(C, G, 2, 128).transpose(3, 0, 1, 2).reshape(128, C * G * 2))
    w_packed = _pack_w(w)

    in_maps = []
    for core in range(NCORES):
        xt = _pack_xt(x[core * BS:(core + 1) * BS])
        in_maps.append({"xt": xt, "w": w_packed, "bias": bias_t})

    res = run_bass_kernel_spmd(nc, in_maps, core_ids=list(range(NCORES)))
    _CACHE["last_results"] = res

    out = np.empty((B, C, IMG, IMG), dtype=np.float32)
    for core in range(NCORES):
        _unpack_out(res.results[core]["out"],
                    out[core * BS:(core + 1) * BS], perm)
    return out


# revision 4
# speedup vs baseline: 1.1071x; 1.0074x over previous
"""Trainium2 Bass kernel for nn_ChannelWisePatchLevelObfuscator.

Per-patch 256x256 dense obfuscation matmul + bias + tanh + channel permutation.
Sharding: data-parallel over batch B=64 across 8 NeuronCores (8 images/core);
weights/biases replicated. Host packs x into a group-sorted, pixel-major layout
so device DMAs are fully contiguous 1 MiB slabs; the device does the matmuls
(PE), bias+tanh (ACT); host scatters patches back to image layout and applies
the channel permutation while assembling the full output.
"""
import sys
import numpy as np

sys.path.insert(0, "/opt/trn_rl_repo")

import concourse.bacc as bacc  # noqa: E402
import concourse.mybir as mybir  # noqa: E402
import concourse.tile as tile  # noqa: E402
from concourse.bass_utils import run_bass_kernel_spmd  # noqa: E402

IMG, C, PS, G, B = 512, 3, 16, 32, 64
NH = NW = IMG // PS          # 32
P2 = PS * PS                 # 256
NCORES = 8
BS = B // NCORES             # 8 images per core
T = BS * NH                  # 256 matmul rows per (c, g)
GB = 4                       # groups per SBUF block (1 MiB tiles)
NGB = G // GB                # 8 blocks per channel

F32 = mybir.dt.float32
MM_DT = mybir.dt.float32r    # matmul input dtype (float32 | float32r)

_g = np.arange(G)[:, None]
_r = np.arange(NH)[None, :]
COLS = (_g - _r) % NW        # (g, r) -> patch column for that group

_CACHE = {}


def _build_nc():
    nc = bacc.Bacc("TRN2", target_bir_lowering=False, debug=False,
                   num_devices=NCORES)
    # slab layouts: [c, gb, 128, free] so each (c, gb) tile load/store is one
    # contiguous 8 KiB descriptor per partition.
    xt = nc.dram_tensor("xt", [C, NGB, 128, GB * 2 * T], MM_DT,
                        kind="ExternalInput")
    w = nc.dram_tensor("w", [C, NGB, 128, GB * 2 * P2], MM_DT,
                       kind="ExternalInput")
    bias = nc.dram_tensor("bias", [128, C * G * 2], F32, kind="ExternalInput")
    out = nc.dram_tensor("out", [C, NGB, 128, GB * 2 * T], F32,
                         kind="ExternalOutput")

    with tile.TileContext(nc) as tc:
        with tc.tile_pool(name="biasp", bufs=1) as bias_pool, \
             tc.tile_pool(name="xtp", bufs=5) as xt_pool, \
             tc.tile_pool(name="wp", bufs=5) as w_pool, \
             tc.tile_pool(name="outp", bufs=4) as out_pool, \
             tc.tile_pool(name="psp", bufs=8, space="PSUM") as ps_pool:
            bias_sb = bias_pool.tile([128, C * G * 2], F32)
            nc.sync.dma_start(bias_sb[:], bias[:, :])
            for c in range(C):
                for gb in range(NGB):
                    xt_t = xt_pool.tile([128, GB * 2 * T], MM_DT)
                    nc.sync.dma_start(xt_t[:], xt[c, gb])
                    w_t = w_pool.tile([128, GB * 2 * P2], MM_DT)
                    nc.sync.dma_start(w_t[:], w[c, gb])
                    out_t = out_pool.tile([128, GB * 2 * T], F32)
                    for gl in range(GB):
                        for oc in range(2):
                            ps = ps_pool.tile([128, T], F32)
                            for kc in range(2):
                                base = (gl * 2 + kc) * P2
                                nc.tensor.matmul(
                                    ps[:],
                                    w_t[:, base + oc * 128: base + oc * 128 + 128],
                                    xt_t[:, (gl * 2 + kc) * T: (gl * 2 + kc + 1) * T],
                                    start=(kc == 0), stop=(kc == 1))
                            bidx = (c * G + gb * GB + gl) * 2 + oc
                            nc.scalar.activation(
                                out_t[:, (gl * 2 + oc) * T: (gl * 2 + oc + 1) * T],
                                ps[:],
                                mybir.ActivationFunctionType.Tanh,
                                bias=bias_sb[:, bidx: bidx + 1],
                                scale=1.0)
                    nc.scalar.dma_start(out[c, gb], out_t[:])
    nc.compile()
    return nc


def _pack_xt(x_shard):
    # (BS, C, 512, 512) -> xt[c, gb, k_lo, (g_lo, kc, t)] slab layout
    xp = x_shard.reshape(BS, C, NH, PS, NW, PS)        # b c r py cl px
    sel = xp[:, :, _r, :, COLS, :]                     # g r b c py px
    xt = sel.transpose(3, 0, 4, 5, 2, 1).reshape(C, G, P2, T)
    # [c, g, p, t] -> [c, gb, k_lo, g_lo, kc, t]
    xt = xt.reshape(C, NGB, GB, 2, 128, T).transpose(0, 1, 4, 2, 3, 5)
    return np.ascontiguousarray(xt.reshape(C, NGB, 128, GB * 2 * T))


def _pack_w(w_full):
    # [c, g, p_in, p_out] -> [c, gb, k_lo, (g_lo, kc, o)]
    w2 = w_full.reshape(C, NGB, GB, 2, 128, P2).transpose(0, 1, 4, 2, 3, 5)
    return np.ascontiguousarray(w2.reshape(C, NGB, 128, GB * 2 * P2))


def _unpack_out(out_dev, dst, perm):
    # out_dev[c, gb, o_lo, (g_lo, oc, t)] -> dst[b, c_final, H, W] + perm
    od = out_dev.reshape(C, NGB, 128, GB, 2, T).transpose(0, 1, 3, 4, 2, 5)
    o = od.reshape(C, G, P2, BS, NH)                   # c g o b r
    src = o.transpose(1, 4, 3, 0, 2).reshape(G, NH, BS, C, PS, PS)
    tmp = np.empty((NH, NW, BS, C, PS, PS), dtype=out_dev.dtype)
    tmp[_r, COLS] = src                                # tmp[r, (g-r)%32] = src[g, r]
    img = tmp.transpose(2, 3, 0, 4, 1, 5).reshape(BS, C, IMG, IMG)
    dst[:] = img[:, perm]


def kernel(x, obfuscation_weights, obfuscation_biases, channel_permutation):
    x = np.ascontiguousarray(x, dtype=np.float32)
    w = np.ascontiguousarray(obfuscation_weights, dtype=np.float32)
    bias = np.asarray(obfuscation_biases, dtype=np.float32)
    perm = np.asarray(channel_permutation, dtype=np.int64)

    if "nc" not in _CACHE:
        _CACHE["nc"] = _build_nc()
    nc = _CACHE["nc"]

    bias_t = np.ascontiguousarray(
        bias.reshape(C, G, 2, 128).transpose(3, 0, 1, 2).reshape(128, C * G * 2))
    w_packed = _pack_w(w)

    in_maps = []
    for core in range(NCORES):
        xt = _pack_xt(x[core * BS:(core + 1) * BS])
        in_maps.append({"xt": xt, "w": w_packed, "bias": bias_t})

    res = run_bass_kernel_spmd(nc, in_maps, core_ids=list(range(NCORES)))
    _CACHE["last_results"] = res

    out = np.empty((B, C, IMG, IMG), dtype=np.float32)
    for core in range(NCORES):
        _unpack_out(res.results[core]["out"],
                    out[core * BS:(core + 1) * BS], perm)
    return out


# revision 5
# speedup vs baseline: 1.8327x; 1.6553x over previous
"""Trainium2 Bass kernel for nn_ChannelWisePatchLevelObfuscator.

Per-patch 256x256 dense obfuscation matmul + bias + tanh + channel permutation.
Sharding: data-parallel over batch B=64 across 8 NeuronCores (8 images/core);
weights/biases replicated. Host packs x into a group-sorted, pixel-major layout
so device DMAs are fully contiguous 1 MiB slabs; the device does the matmuls
(PE), bias+tanh (ACT); host scatters patches back to image layout and applies
the channel permutation while assembling the full output.
"""
import sys
import numpy as np

sys.path.insert(0, "/opt/trn_rl_repo")

import concourse.bacc as bacc  # noqa: E402
import concourse.mybir as mybir  # noqa: E402
import concourse.tile as tile  # noqa: E402
from concourse.bass_utils import run_bass_kernel_spmd  # noqa: E402

IMG, C, PS, G, B = 512, 3, 16, 32, 64
NH = NW = IMG // PS          # 32
P2 = PS * PS                 # 256
NCORES = 8
BS = B // NCORES             # 8 images per core
T = BS * NH                  # 256 matmul rows per (c, g)
GB = 4                       # groups per SBUF block (1 MiB tiles)
NGB = G // GB                # 8 blocks per channel

F32 = mybir.dt.float32
MM_DT = mybir.dt.float16     # matmul input dtype (float32|float32r|float16)
NP_MM = np.float16 if MM_DT == mybir.dt.float16 else np.float32

_g = np.arange(G)[:, None]
_r = np.arange(NH)[None, :]
COLS = (_g - _r) % NW        # (g, r) -> patch column for that group

_CACHE = {}


def _build_nc():
    nc = bacc.Bacc("TRN2", target_bir_lowering=False, debug=False,
                   num_devices=NCORES)
    # slab layouts: [c, gb, 128, free] so each (c, gb) tile load/store is one
    # contiguous 8 KiB descriptor per partition.
    xt = nc.dram_tensor("xt", [C, NGB, 128, GB * 2 * T], MM_DT,
                        kind="ExternalInput")
    w = nc.dram_tensor("w", [C, NGB, 128, GB * 2 * P2], MM_DT,
                       kind="ExternalInput")
    bias = nc.dram_tensor("bias", [128, C * G * 2], F32, kind="ExternalInput")
    out = nc.dram_tensor("out", [C, NGB, 128, GB * 2 * T], F32,
                         kind="ExternalOutput")

    with tile.TileContext(nc) as tc:
        with tc.tile_pool(name="biasp", bufs=1) as bias_pool, \
             tc.tile_pool(name="xtp", bufs=5) as xt_pool, \
             tc.tile_pool(name="wp", bufs=5) as w_pool, \
             tc.tile_pool(name="outp", bufs=4) as out_pool, \
             tc.tile_pool(name="psp", bufs=8, space="PSUM") as ps_pool:
            bias_sb = bias_pool.tile([128, C * G * 2], F32)
            nc.sync.dma_start(bias_sb[:], bias[:, :])
            for c in range(C):
                for gb in range(NGB):
                    xt_t = xt_pool.tile([128, GB * 2 * T], MM_DT)
                    nc.sync.dma_start(xt_t[:], xt[c, gb])
                    w_t = w_pool.tile([128, GB * 2 * P2], MM_DT)
                    nc.sync.dma_start(w_t[:], w[c, gb])
                    out_t = out_pool.tile([128, GB * 2 * T], F32)
                    for gl in range(GB):
                        for oc in range(2):
                            ps = ps_pool.tile([128, T], F32)
                            for kc in range(2):
                                base = (gl * 2 + kc) * P2
                                nc.tensor.matmul(
                                    ps[:],
                                    w_t[:, base + oc * 128: base + oc * 128 + 128],
                                    xt_t[:, (gl * 2 + kc) * T: (gl * 2 + kc + 1) * T],
                                    start=(kc == 0), stop=(kc == 1))
                            bidx = (c * G + gb * GB + gl) * 2 + oc
                            nc.scalar.activation(
                                out_t[:, (gl * 2 + oc) * T: (gl * 2 + oc + 1) * T],
                                ps[:],
                                mybir.ActivationFunctionType.Tanh,
                                bias=bias_sb[:, bidx: bidx + 1],
                                scale=1.0)
                    nc.scalar.dma_start(out[c, gb], out_t[:])
    nc.compile()
    return nc


def _pack_xt(x_shard):
    # (BS, C, 512, 512) -> xt[c, gb, k_lo, (g_lo, kc, t)] slab layout
    xp = x_shard.reshape(BS, C, NH, PS, NW, PS)        # b c r py cl px
    sel = xp[:, :, _r, :, COLS, :]                     # g r b c py px
    xt = sel.transpose(3, 0, 4, 5, 2, 1).reshape(C, G, P2, T).astype(NP_MM)
    # [c, g, p, t] -> [c, gb, k_lo, g_lo, kc, t]
    xt = xt.reshape(C, NGB, GB, 2, 128, T).transpose(0, 1, 4, 2, 3, 5)
    return np.ascontiguousarray(xt.reshape(C, NGB, 128, GB * 2 * T))


def _pack_w(w_full):
    # [c, g, p_in, p_out] -> [c, gb, k_lo, (g_lo, kc, o)]
    w2 = w_full.astype(NP_MM).reshape(C, NGB, GB, 2, 128, P2).transpose(0, 1, 4, 2, 3, 5)
    return np.ascontiguousarray(w2.reshape(C, NGB, 128, GB * 2 * P2))


def _unpack_out(out_dev, dst, perm):
    # out_dev[c, gb, o_lo, (g_lo, oc, t)] -> dst[b, c_final, H, W] + perm
    od = out_dev.reshape(C, NGB, 128, GB, 2, T).transpose(0, 1, 3, 4, 2, 5)
    o = od.reshape(C, G, P2, BS, NH)                   # c g o b r
    src = o.transpose(1, 4, 3, 0, 2).reshape(G, NH, BS, C, PS, PS)
    tmp = np.empty((NH, NW, BS, C, PS, PS), dtype=out_dev.dtype)
    tmp[_r, COLS] = src                                # tmp[r, (g-r)%32] = src[g, r]
    img = tmp.transpose(2, 3, 0, 4, 1, 5).reshape(BS, C, IMG, IMG)
    dst[:] = img[:, perm]


def kernel(x, obfuscation_weights, obfuscation_biases, channel_permutation):
    x = np.ascontiguousarray(x, dtype=np.float32)
    w = np.ascontiguousarray(obfuscation_weights, dtype=np.float32)
    bias = np.asarray(obfuscation_biases, dtype=np.float32)
    perm = np.asarray(channel_permutation, dtype=np.int64)

    if "nc" not in _CACHE:
        _CACHE["nc"] = _build_nc()
    nc = _CACHE["nc"]

    bias_t = np.ascontiguousarray(
        bias.reshape(C, G, 2, 128).transpose(3, 0, 1, 2).reshape(128, C * G * 2))
    w_packed = _pack_w(w)

    in_maps = []
    for core in range(NCORES):
        xt = _pack_xt(x[core * BS:(core + 1) * BS])
        in_maps.append({"xt": xt, "w": w_packed, "bias": bias_t})

    res = run_bass_kernel_spmd(nc, in_maps, core_ids=list(range(NCORES)))
    _CACHE["last_results"] = res

    out = np.empty((B, C, IMG, IMG), dtype=np.float32)
    for core in range(NCORES):
        _unpack_out(res.results[core]["out"],
                    out[core * BS:(core + 1) * BS], perm)
    return out


# revision 6
# speedup vs baseline: 2.0240x; 1.1044x over previous
"""Trainium2 Bass kernel for nn_ChannelWisePatchLevelObfuscator.

Per-patch 256x256 dense obfuscation matmul + bias + tanh + channel permutation.
Sharding: data-parallel over batch B=64 across 8 NeuronCores (8 images/core);
weights/biases replicated. Host packs x into a group-sorted, pixel-major layout
so device DMAs are fully contiguous 1 MiB slabs; the device does the matmuls
(PE), bias+tanh (ACT); host scatters patches back to image layout and applies
the channel permutation while assembling the full output.
"""
import sys
import numpy as np

sys.path.insert(0, "/opt/trn_rl_repo")

import concourse.bacc as bacc  # noqa: E402
import concourse.mybir as mybir  # noqa: E402
import concourse.tile as tile  # noqa: E402
from concourse.bass_utils import run_bass_kernel_spmd  # noqa: E402

IMG, C, PS, G, B = 512, 3, 16, 32, 64
NH = NW = IMG // PS          # 32
P2 = PS * PS                 # 256
NCORES = 8
BS = B // NCORES             # 8 images per core
T = BS * NH                  # 256 matmul rows per (c, g)
GB = 4                       # groups per SBUF block (1 MiB tiles)
NGB = G // GB                # 8 blocks per channel

F32 = mybir.dt.float32
MM_DT = mybir.dt.float16     # matmul input dtype (float32|float32r|float16)
NP_MM = np.float16 if MM_DT == mybir.dt.float16 else np.float32
OUT_DT = mybir.dt.float16    # device store dtype; host upcasts to fp32

_g = np.arange(G)[:, None]
_r = np.arange(NH)[None, :]
COLS = (_g - _r) % NW        # (g, r) -> patch column for that group

_CACHE = {}


def _build_nc():
    nc = bacc.Bacc("TRN2", target_bir_lowering=False, debug=False,
                   num_devices=NCORES)
    # slab layouts: [c, gb, 128, free] so each (c, gb) tile load/store is one
    # contiguous 8 KiB descriptor per partition.
    xt = nc.dram_tensor("xt", [C, NGB, 128, GB * 2 * T], MM_DT,
                        kind="ExternalInput")
    w = nc.dram_tensor("w", [C, NGB, 128, GB * 2 * P2], MM_DT,
                       kind="ExternalInput")
    bias = nc.dram_tensor("bias", [128, C * G * 2], F32, kind="ExternalInput")
    out = nc.dram_tensor("out", [C, NGB, 128, GB * 2 * T], OUT_DT,
                         kind="ExternalOutput")

    with tile.TileContext(nc) as tc:
        with tc.tile_pool(name="biasp", bufs=1) as bias_pool, \
             tc.tile_pool(name="xtp", bufs=5) as xt_pool, \
             tc.tile_pool(name="wp", bufs=5) as w_pool, \
             tc.tile_pool(name="outp", bufs=4) as out_pool, \
             tc.tile_pool(name="psp", bufs=8, space="PSUM") as ps_pool:
            bias_sb = bias_pool.tile([128, C * G * 2], F32)
            nc.sync.dma_start(bias_sb[:], bias[:, :])
            for c in range(C):
                for gb in range(NGB):
                    xt_t = xt_pool.tile([128, GB * 2 * T], MM_DT)
                    nc.sync.dma_start(xt_t[:], xt[c, gb])
                    w_t = w_pool.tile([128, GB * 2 * P2], MM_DT)
                    nc.sync.dma_start(w_t[:], w[c, gb])
                    out_t = out_pool.tile([128, GB * 2 * T], OUT_DT)
                    for gl in range(GB):
                        for oc in range(2):
                            ps = ps_pool.tile([128, T], F32)
                            for kc in range(2):
                                base = (gl * 2 + kc) * P2
                                nc.tensor.matmul(
                                    ps[:],
                                    w_t[:, base + oc * 128: base + oc * 128 + 128],
                                    xt_t[:, (gl * 2 + kc) * T: (gl * 2 + kc + 1) * T],
                                    start=(kc == 0), stop=(kc == 1))
                            bidx = (c * G + gb * GB + gl) * 2 + oc
                            nc.scalar.activation(
                                out_t[:, (gl * 2 + oc) * T: (gl * 2 + oc + 1) * T],
                                ps[:],
                                mybir.ActivationFunctionType.Tanh,
                                bias=bias_sb[:, bidx: bidx + 1],
                                scale=1.0)
                    nc.scalar.dma_start(out[c, gb], out_t[:])
    nc.compile()
    return nc


def _pack_xt(x_shard):
    # (BS, C, 512, 512) -> xt[c, gb, k_lo, (g_lo, kc, t)] slab layout
    xp = x_shard.reshape(BS, C, NH, PS, NW, PS)        # b c r py cl px
    sel = xp[:, :, _r, :, COLS, :]                     # g r b c py px
    xt = sel.transpose(3, 0, 4, 5, 2, 1).reshape(C, G, P2, T).astype(NP_MM)
    # [c, g, p, t] -> [c, gb, k_lo, g_lo, kc, t]
    xt = xt.reshape(C, NGB, GB, 2, 128, T).transpose(0, 1, 4, 2, 3, 5)
    return np.ascontiguousarray(xt.reshape(C, NGB, 128, GB * 2 * T))


def _pack_w(w_full):
    # [c, g, p_in, p_out] -> [c, gb, k_lo, (g_lo, kc, o)]
    w2 = w_full.astype(NP_MM).reshape(C, NGB, GB, 2, 128, P2).transpose(0, 1, 4, 2, 3, 5)
    return np.ascontiguousarray(w2.reshape(C, NGB, 128, GB * 2 * P2))


def _unpack_out(out_dev, dst, perm):
    # out_dev[c, gb, o_lo, (g_lo, oc, t)] -> dst[b, c_final, H, W] + perm
    od = out_dev.astype(np.float32).reshape(C, NGB, 128, GB, 2, T).transpose(0, 1, 3, 4, 2, 5)
    o = od.reshape(C, G, P2, BS, NH)                   # c g o b r
    src = o.transpose(1, 4, 3, 0, 2).reshape(G, NH, BS, C, PS, PS)
    tmp = np.empty((NH, NW, BS, C, PS, PS), dtype=out_dev.dtype)
    tmp[_r, COLS] = src                                # tmp[r, (g-r)%32] = src[g, r]
    img = tmp.transpose(2, 3, 0, 4, 1, 5).reshape(BS, C, IMG, IMG)
    dst[:] = img[:, perm]


def kernel(x, obfuscation_weights, obfuscation_biases, channel_permutation):
    x = np.ascontiguousarray(x, dtype=np.float32)
    w = np.ascontiguousarray(obfuscation_weights, dtype=np.float32)
    bias = np.asarray(obfuscation_biases, dtype=np.float32)
    perm = np.asarray(channel_permutation, dtype=np.int64)

    if "nc" not in _CACHE:
        _CACHE["nc"] = _build_nc()
    nc = _CACHE["nc"]

    bias_t = np.ascontiguousarray(
        bias.reshape(C, G, 2, 128).transpose(3, 0, 1, 2).reshape(128, C * G * 2))
    w_packed = _pack_w(w)

    in_maps = []
    for core in range(NCORES):
        xt = _pack_xt(x[core * BS:(core + 1) * BS])
        in_maps.append({"xt": xt, "w": w_packed, "bias": bias_t})

    res = run_bass_kernel_spmd(nc, in_maps, core_ids=list(range(NCORES)))
    _CACHE["last_results"] = res

    out = np.empty((B, C, IMG, IMG), dtype=np.float32)
    for core in range(NCORES):
        _unpack_out(res.results[core]["out"],
                    out[core * BS:(core + 1) * BS], perm)
    return out


# revision 7
# speedup vs baseline: 2.1762x; 1.0752x over previous
"""Trainium2 Bass kernel for nn_ChannelWisePatchLevelObfuscator.

Per-patch 256x256 dense obfuscation matmul + bias + tanh + channel permutation.
Sharding: data-parallel over batch B=64 across 8 NeuronCores (8 images/core);
weights/biases replicated. Host packs x into a group-sorted, pixel-major layout
so device DMAs are fully contiguous 1 MiB slabs; the device does the matmuls
(PE), bias+tanh (ACT); host scatters patches back to image layout and applies
the channel permutation while assembling the full output.
"""
import sys
import numpy as np

sys.path.insert(0, "/opt/trn_rl_repo")

import concourse.bacc as bacc  # noqa: E402
import concourse.mybir as mybir  # noqa: E402
import concourse.tile as tile  # noqa: E402
from concourse.bass_utils import run_bass_kernel_spmd  # noqa: E402

IMG, C, PS, G, B = 512, 3, 16, 32, 64
NH = NW = IMG // PS          # 32
P2 = PS * PS                 # 256
NCORES = 8
BS = B // NCORES             # 8 images per core
T = BS * NH                  # 256 matmul rows per (c, g)
GB = 4                       # groups per SBUF block (1 MiB tiles)
NGB = G // GB                # 8 blocks per channel

F32 = mybir.dt.float32
MM_DT = mybir.dt.float16     # matmul input dtype (float32|float32r|float16)
NP_MM = np.float16 if MM_DT == mybir.dt.float16 else np.float32
OUT_DT = mybir.dt.float16    # device store dtype; host upcasts to fp32

_g = np.arange(G)[:, None]
_r = np.arange(NH)[None, :]
COLS = (_g - _r) % NW        # (g, r) -> patch column for that group

_CACHE = {}


def _build_nc():
    nc = bacc.Bacc("TRN2", target_bir_lowering=False, debug=False,
                   num_devices=NCORES)
    # slab layouts: [c, gb, 128, free] so each (c, gb) tile load/store is one
    # contiguous 8 KiB descriptor per partition.
    xt = nc.dram_tensor("xt", [C, NGB, 128, GB * 2 * T], MM_DT,
                        kind="ExternalInput")
    w = nc.dram_tensor("w", [C, NGB, 128, GB * 2 * P2], MM_DT,
                       kind="ExternalInput")
    bias = nc.dram_tensor("bias", [128, C * G * 2], F32, kind="ExternalInput")
    out = nc.dram_tensor("out", [C, NGB, 128, GB * 2 * T], OUT_DT,
                         kind="ExternalOutput")

    with tile.TileContext(nc) as tc:
        with tc.tile_pool(name="biasp", bufs=1) as bias_pool, \
             tc.tile_pool(name="xtp", bufs=8) as xt_pool, \
             tc.tile_pool(name="wp", bufs=8) as w_pool, \
             tc.tile_pool(name="outp", bufs=6) as out_pool, \
             tc.tile_pool(name="psp", bufs=8, space="PSUM") as ps_pool:
            bias_sb = bias_pool.tile([128, C * G * 2], F32)
            nc.sync.dma_start(bias_sb[:], bias[:, :])
            for c in range(C):
                for gb in range(NGB):
                    xt_t = xt_pool.tile([128, GB * 2 * T], MM_DT)
                    nc.sync.dma_start(xt_t[:], xt[c, gb])
                    w_t = w_pool.tile([128, GB * 2 * P2], MM_DT)
                    nc.sync.dma_start(w_t[:], w[c, gb])
                    out_t = out_pool.tile([128, GB * 2 * T], OUT_DT)
                    for gl in range(GB):
                        for oc in range(2):
                            ps = ps_pool.tile([128, T], F32)
                            for kc in range(2):
                                base = (gl * 2 + kc) * P2
                                nc.tensor.matmul(
                                    ps[:],
                                    w_t[:, base + oc * 128: base + oc * 128 + 128],
                                    xt_t[:, (gl * 2 + kc) * T: (gl * 2 + kc + 1) * T],
                                    start=(kc == 0), stop=(kc == 1))
                            bidx = (c * G + gb * GB + gl) * 2 + oc
                            nc.scalar.activation(
                                out_t[:, (gl * 2 + oc) * T: (gl * 2 + oc + 1) * T],
                                ps[:],
                                mybir.ActivationFunctionType.Tanh,
                                bias=bias_sb[:, bidx: bidx + 1],
                                scale=1.0)
                    nc.scalar.dma_start(out[c, gb], out_t[:])
    nc.compile()
    return nc


def _pack_xt(x_shard):
    # (BS, C, 512, 512) -> xt[c, gb, k_lo, (g_lo, kc, t)] slab layout
    xp = x_shard.reshape(BS, C, NH, PS, NW, PS)        # b c r py cl px
    sel = xp[:, :, _r, :, COLS, :]                     # g r b c py px
    xt = sel.transpose(3, 0, 4, 5, 2, 1).reshape(C, G, P2, T).astype(NP_MM)
    # [c, g, p, t] -> [c, gb, k_lo, g_lo, kc, t]
    xt = xt.reshape(C, NGB, GB, 2, 128, T).transpose(0, 1, 4, 2, 3, 5)
    return np.ascontiguousarray(xt.reshape(C, NGB, 128, GB * 2 * T))


def _pack_w(w_full):
    # [c, g, p_in, p_out] -> [c, gb, k_lo, (g_lo, kc, o)]
    w2 = w_full.astype(NP_MM).reshape(C, NGB, GB, 2, 128, P2).transpose(0, 1, 4, 2, 3, 5)
    return np.ascontiguousarray(w2.reshape(C, NGB, 128, GB * 2 * P2))


def _unpack_out(out_dev, dst, perm):
    # out_dev[c, gb, o_lo, (g_lo, oc, t)] -> dst[b, c_final, H, W] + perm
    od = out_dev.astype(np.float32).reshape(C, NGB, 128, GB, 2, T).transpose(0, 1, 3, 4, 2, 5)
    o = od.reshape(C, G, P2, BS, NH)                   # c g o b r
    src = o.transpose(1, 4, 3, 0, 2).reshape(G, NH, BS, C, PS, PS)
    tmp = np.empty((NH, NW, BS, C, PS, PS), dtype=out_dev.dtype)
    tmp[_r, COLS] = src                                # tmp[r, (g-r)%32] = src[g, r]
    img = tmp.transpose(2, 3, 0, 4, 1, 5).reshape(BS, C, IMG, IMG)
    dst[:] = img[:, perm]


def kernel(x, obfuscation_weights, obfuscation_biases, channel_permutation):
    x = np.ascontiguousarray(x, dtype=np.float32)
    w = np.ascontiguousarray(obfuscation_weights, dtype=np.float32)
    bias = np.asarray(obfuscation_biases, dtype=np.float32)
    perm = np.asarray(channel_permutation, dtype=np.int64)

    if "nc" not in _CACHE:
        _CACHE["nc"] = _build_nc()
    nc = _CACHE["nc"]

    bias_t = np.ascontiguousarray(
        bias.reshape(C, G, 2, 128).transpose(3, 0, 1, 2).reshape(128, C * G * 2))
    w_packed = _pack_w(w)

    in_maps = []
    for core in range(NCORES):
        xt = _pack_xt(x[core * BS:(core + 1) * BS])
        in_maps.append({"xt": xt, "w": w_packed, "bias": bias_t})

    res = run_bass_kernel_spmd(nc, in_maps, core_ids=list(range(NCORES)))
    _CACHE["last_results"] = res

    out = np.empty((B, C, IMG, IMG), dtype=np.float32)
    for core in range(NCORES):
        _unpack_out(res.results[core]["out"],
                    out[core * BS:(core + 1) * BS], perm)
    return out
